# revision 49
# baseline (speedup 1.0000x reference)
"""Trainium2 Bass kernel for nn_ContrastiveLoss (B=4096, D=1024, 8 cores).

loss = mean over [B,B] of
    labels*(1-sim0) + (1-labels)*relu(sim0-0.5)
  + labels*(1-sim1) + (1-labels)*relu(sim1-0.5)
where sim_k = cos_sim(fc_feats_k[i], textual_features[j]).

Strategy (data-parallel over rows, t replicated -- no collectives):
  * Each of the 8 cores gets a 512-row slice of fc_feats_0/1 and labels
    (bf16 from the host), plus the FULL textual_features (bf16).
  * t is normalized on-chip (x64 fp8 scale); transposes are done by the
    XBAR DMA engine (dma_start_transpose) on bf16 tiles -- no PE
    transposes, no evacuation copies; a single cast pass produces fp8.
  * f0/f1 are NOT normalized: raw bf16 rows are XBAR-transposed and
    cast to fp8; 1/||f_i|| is applied as a per-partition ACT scale in
    the relu pass and factored out of the bilinear sums algebraically.
  * Per S-tile [128,1024] (= 2 j-chunks, one psum tile per f):
      ACT:  r = relu(rin_f * S - 0.5), accum -> racc
      DVE:  accum L * S_raw           -> pacc   (x rin_f in finisher)
      DVE:  w = r0 + r1; accum (w-2)*L -> qacc  (folds C and 2*sum(L))
  * total_core = sum_i [ A - q - rin0*p0 - rin1*p1 ]; host sums / B^2.

Self-contained: hardcodes shapes; only needs concourse + ml_dtypes.
"""

import os
import sys

import numpy as np

B = 4096
D = 1024
NCORES = 8
ROWS = B // NCORES          # 512 rows of f0/f1/labels per core
IT = ROWS // 128            # 4 i-tiles per core
KS = D // 128               # 8 k-subtiles (contraction)
JQ = 4                      # phase-B chunk = 2 slots = 1024 S columns
NSLOT = 8                   # t row-slots of 512 (full t on every core)
MARGIN = 0.5
EPS = 1e-8
TN_SCALE = 64.0             # fp8 scale on normalized t rows

_CACHE = {}

# how many of the 8 t-slot fp8 casts run on ACT (rest on DVE)
# (note: tensor_tensor_reduce fails at runtime on this HW path; squares
# use DVE scalar_tensor_tensor instead)
CASTS_ON_ACT = int(os.environ.get("KERNEL_CASTS_ON_ACT", "6"))


def _import_concourse():
    try:
        import concourse.bass  # noqa: F401
    except ImportError:
        for p in ("/opt/trn_rl_repo", "/root/.axon_site/_ro/trn_rl_repo"):
            if os.path.isdir(p) and p not in sys.path:
                sys.path.insert(0, p)
        import concourse.bass  # noqa: F401


def _build_nc():
    _import_concourse()
    import concourse.bass as bass  # noqa: F401
    import concourse.mybir as mybir
    import concourse.tile as tile
    from concourse import bacc

    f32 = mybir.dt.float32
    bf16 = mybir.dt.bfloat16
    fp8 = mybir.dt.float8e4
    AF = mybir.ActivationFunctionType
    OP = mybir.AluOpType
    AX = mybir.AxisListType
    DR = mybir.MatmulPerfMode.DoubleRow

    nc = bacc.Bacc(
        "TRN2",
        target_bir_lowering=False,
        debug=False,
        num_devices=NCORES,
    )

    f0_d = nc.dram_tensor("f0", [ROWS, D], bf16, kind="ExternalInput").ap()
    f1_d = nc.dram_tensor("f1", [ROWS, D], bf16, kind="ExternalInput").ap()
    tx_d = nc.dram_tensor("tx", [B, D], bf16, kind="ExternalInput").ap()
    lab_d = nc.dram_tensor("lab", [ROWS, B], bf16, kind="ExternalInput").ap()
    out_d = nc.dram_tensor("outv", [128, 1], f32, kind="ExternalOutput").ap()

    with tile.TileContext(nc) as tc:
        with (
            tc.tile_pool(name="constp", bufs=1) as constp,
            tc.tile_pool(name="natp", bufs=5) as natp,
            tc.tile_pool(name="tnbp", bufs=2) as tnbp,
            tc.tile_pool(name="stagep", bufs=3) as stagep,
            tc.tile_pool(name="sqp", bufs=2) as sqp,
            tc.tile_pool(name="small", bufs=6) as small,
            tc.tile_pool(name="fTp", bufs=1) as fTp,
            tc.tile_pool(name="tnTp", bufs=1) as tnTp,
            tc.tile_pool(name="labp", bufs=4) as labp,
            tc.tile_pool(name="rbufp", bufs=4) as rbufp,
            tc.tile_pool(name="scrp", bufs=4) as scrp,
            tc.tile_pool(name="accp", bufs=1) as accp,
            tc.tile_pool(name="mpsum", bufs=4, space="PSUM") as mpsum,
        ):
            negmargin = constp.tile([128, 1], f32)
            nc.gpsimd.memset(negmargin, -MARGIN)

            # persistent per-(f,ic) inverse norms, col = f*IT + ic
            rinp = accp.tile([128, 2 * IT], f32)
            # accumulators, each column written exactly once
            racc = accp.tile([128, 2 * IT * JQ], f32)   # sum relu
            pacc = accp.tile([128, 2 * IT * JQ], f32)   # sum L*Sraw
            qacc = accp.tile([128, IT * JQ], f32)       # sum (w-2)*L

            # ---- input DMAs. t/f stream on the Pool (gpsimd) ring in slot
            # order (the ring's depth-2 + tile-pool recycling paces it);
            # labels ride the Sync ring FIRST (they complete before the
            # first XBAR transpose needs that ring). No compute queue
            # hosts DMA triggers. ----
            Lbs = []
            for ic in range(IT):
                Lb = labp.tile([128, B], bf16, tag="Lb", name=f"Lb_{ic}")
                nc.sync.dma_start(Lb, lab_d[ic * 128:(ic + 1) * 128, :])
                Lbs.append(Lb)

            def t_dma(s):
                t = natp.tile([128, 4, D], bf16, tag="nat", name=f"tnat_{s}")
                nc.gpsimd.dma_start(
                    t, tx_d[s * 512:(s + 1) * 512, :].rearrange(
                        "(r p) d -> p r d", p=128))
                return t

            tnat = {s: t_dma(s) for s in range(2)}
            fnat = []
            for f, src in enumerate((f0_d, f1_d)):
                natb = natp.tile([128, IT, D], bf16, tag="nat",
                                 name=f"fnat_{f}")
                nc.gpsimd.dma_start(
                    natb, src.rearrange("(r p) d -> p r d", p=128))
                fnat.append(natb)
            for s in range(2, NSLOT):
                tnat[s] = t_dma(s)

            def norm_smalls(ssq, n, scale_mul, dst):
                """[128,n] batched: dst = scale_mul / max(sqrt(ssq), EPS)."""
                nrm = small.tile([128, n], f32, tag="nrm",
                                 name=f"nrm_{dst.tensor.name}_{dst.offset}")
                nc.scalar.activation(nrm, ssq, AF.Sqrt)
                nc.vector.tensor_scalar_max(nrm, nrm, EPS)
                nc.vector.reciprocal(dst, nrm)
                nc.vector.tensor_scalar_mul(dst, dst, scale_mul)

            def squares(natb, it, ssq_col, name, on_act):
                """Sum-of-squares of natb[:, it, :] -> ssq_col. ACT Square
                stalls ~1.2us on its accumulator between instructions, so
                early tiles run on DVE (idle then) and later ones on ACT."""
                sqs = sqp.tile([128, D], bf16, tag="sq", name=name)
                if on_act:
                    nc.scalar.activation(sqs, natb[:, it, :], AF.Square,
                                         accum_out=ssq_col)
                else:
                    nc.vector.scalar_tensor_tensor(
                        out=sqs, in0=natb[:, it, :], scalar=1.0,
                        in1=natb[:, it, :], op0=OP.bypass, op1=OP.mult,
                        accum_out=ssq_col)

            def cast_stage(dst8, stage32, on_act):
                """bf16 [128,32,128] xbar stage -> fp8 [128,8,512] with the
                (r ks) -> ks r layout fix folded into the strided read."""
                src = stage32[:, :, :].rearrange("p (r ks) j -> p ks r j",
                                                 ks=KS)
                dst = dst8.rearrange("p ks (r j) -> p ks r j", j=128)
                (nc.scalar.copy if on_act else nc.vector.tensor_copy)(dst, src)

            tnT = tnTp.tile([128, NSLOT * KS, 512], fp8)

            def t_prep(s):
                """Normalize + xbar-transpose + fp8-cast t slot s into tnT."""
                if s not in tnat:
                    tnat[s] = t_dma(s)
                natb = tnat.pop(s)
                ssqs = small.tile([128, 4], f32, tag="ssq", name=f"ssqt_{s}")
                for it in range(4):
                    squares(natb, it, ssqs[:, it:it + 1], f"tsq_{s}_{it}",
                            on_act=(s >= 4))
                rint = small.tile([128, 4], f32, tag="rint", name=f"rint_{s}")
                norm_smalls(ssqs, 4, TN_SCALE, rint)
                tnb = tnbp.tile([128, 4, D], bf16, tag="tnb", name=f"tnb_{s}")
                for it in range(4):
                    nc.vector.tensor_scalar_mul(
                        tnb[:, it, :], natb[:, it, :], rint[:, it:it + 1])
                tstage = stagep.tile([128, 4 * KS, 128], bf16, tag="stage",
                                     name=f"tstage_{s}")
                nc.sync.dma_start_transpose(tstage, tnb[:, :, :])
                # early slots cast on DVE (ACT is busy with squares then);
                # later slots on ACT to balance total load
                cast_stage(tnT[:, s * KS:(s + 1) * KS, :], tstage,
                           on_act=(s >= 3))

            # ---- t slots 0,1 first, then f0/f1 ----
            t_prep(0)
            t_prep(1)
            fTs = []
            for f in range(2):
                natb = fnat[f]
                ssqf = small.tile([128, IT], f32, tag="ssq", name=f"ssqf_{f}")
                for it in range(IT):
                    squares(natb, it, ssqf[:, it:it + 1], f"fsq_{f}_{it}",
                            on_act=True)
                norm_smalls(ssqf, IT, 1.0 / TN_SCALE,
                            rinp[:, f * IT:(f + 1) * IT])
                fstage = stagep.tile([128, IT * KS, 128], bf16, tag="stage",
                                     name=f"fstage_{f}")
                nc.sync.dma_start_transpose(fstage, natb[:, :, :])
                fT = fTp.tile([128, KS, ROWS], fp8, name=f"fT_{f}")
                cast_stage(fT, fstage, on_act=(f == 1))
                fTs.append(fT)

            # ---- all slot prep emitted up front: ACT/DVE run it densely
            # from t=0 instead of pacing it behind each jq's consumption ----
            for s in range(2, NSLOT):
                t_prep(s)
            for jq in range(JQ):
                jsl = slice(jq * 1024, (jq + 1) * 1024)
                for ic in range(IT):
                    isl = slice(ic * 128, (ic + 1) * 128)
                    Lb = Lbs[ic]
                    rbufs = []
                    for f in range(2):
                        fT = fTs[f]
                        ps = mpsum.tile([128, 1024], f32, tag="ps",
                                        name=f"ps_{jq}_{ic}_{f}")
                        for jh in range(2):
                            jc = jq * 2 + jh
                            for k2 in range(KS // 2):
                                ksl = slice(jc * KS + 2 * k2,
                                            jc * KS + 2 * k2 + 2)
                                nc.tensor.matmul(
                                    ps[:, jh * 512:(jh + 1) * 512],
                                    fT[:, 2 * k2:2 * k2 + 2, isl],
                                    tnT[:, ksl, :], perf_mode=DR,
                                    start=(k2 == 0), stop=(k2 == KS // 2 - 1),
                                )
                        g = f * IT + ic
                        rsc = rinp[:, g:g + 1]
                        rb = rbufp.tile([128, 1024], bf16, tag="rb",
                                        name=f"rb_{jq}_{ic}_{f}")
                        pi = (ic * JQ + jq) * 2 + f
                        nc.scalar.activation(rb, ps, AF.Relu, bias=negmargin,
                                             scale=rsc,
                                             accum_out=racc[:, pi:pi + 1])
                        rbufs.append(rb)
                        scr = scrp.tile([128, 1024], bf16, tag="scr",
                                        name=f"scr_{jq}_{ic}_{f}")
                        qi = g * JQ + jq
                        nc.vector.scalar_tensor_tensor(
                            out=scr, in0=ps, scalar=1.0, in1=Lb[:, jsl],
                            op0=OP.bypass, op1=OP.mult,
                            accum_out=pacc[:, qi:qi + 1])
                    # w = r0 + r1 via stt (all-bf16 -> 4x); (w-2)*L accum
                    w = rbufp.tile([128, 1024], bf16, tag="w",
                                   name=f"w_{jq}_{ic}")
                    nc.vector.scalar_tensor_tensor(
                        out=w, in0=rbufs[0], scalar=0.0, in1=rbufs[1],
                        op0=OP.add, op1=OP.add)
                    scr2 = scrp.tile([128, 1024], bf16, tag="scr2",
                                     name=f"scr2_{jq}_{ic}")
                    wi = ic * JQ + jq
                    nc.vector.scalar_tensor_tensor(
                        out=scr2, in0=w, scalar=2.0, in1=Lb[:, jsl],
                        op0=OP.subtract, op1=OP.mult,
                        accum_out=qacc[:, wi:wi + 1])


            # ---- finisher ----
            pred = accp.tile([128, 2 * IT], f32)
            for g in range(2 * IT):
                nc.vector.reduce_sum(pred[:, g:g + 1],
                                     pacc[:, g * JQ:(g + 1) * JQ], axis=AX.X)
            scaled = accp.tile([128, 2 * IT], f32)
            nc.vector.tensor_mul(scaled, pred, rinp)
            ps_tot = small.tile([128, 1], f32, tag="fin", name="ps_tot")
            nc.vector.reduce_sum(ps_tot, scaled, axis=AX.X)
            ar = small.tile([128, 1], f32, tag="fin", name="ar")
            nc.vector.reduce_sum(ar, racc, axis=AX.X)
            qr = small.tile([128, 1], f32, tag="fin", name="qr")
            nc.vector.reduce_sum(qr, qacc, axis=AX.X)
            tmp = small.tile([128, 1], f32, tag="fin", name="tmp")
            nc.vector.tensor_sub(tmp, ar, qr)
            ov = small.tile([128, 1], f32, tag="fin", name="ov")
            nc.vector.tensor_sub(ov, tmp, ps_tot)
            nc.sync.dma_start(out_d, ov)

    nc.compile()
    return nc


def _get_nc():
    if "nc" not in _CACHE:
        _CACHE["nc"] = _build_nc()
    return _CACHE["nc"]


def _make_in_maps(fc_feats_0, fc_feats_1, textual_features, labels):
    import ml_dtypes
    b16 = ml_dtypes.bfloat16
    tx = np.ascontiguousarray(textual_features).astype(b16)
    in_maps = []
    for c in range(NCORES):
        sl = slice(c * ROWS, (c + 1) * ROWS)
        in_maps.append({
            "f0": np.ascontiguousarray(fc_feats_0[sl]).astype(b16),
            "f1": np.ascontiguousarray(fc_feats_1[sl]).astype(b16),
            "tx": tx,
            "lab": np.ascontiguousarray(labels[sl]).astype(b16),
        })
    return in_maps


def run(fc_feats_0, fc_feats_1, textual_features, labels, trace=False):
    """Run on 8 NeuronCores; returns (loss_scalar, BassKernelResults)."""
    _import_concourse()
    from concourse.bass_utils import run_bass_kernel_spmd

    nc = _get_nc()
    in_maps = _make_in_maps(np.asarray(fc_feats_0), np.asarray(fc_feats_1),
                            np.asarray(textual_features), np.asarray(labels))
    res = run_bass_kernel_spmd(nc, in_maps, list(range(NCORES)), trace=trace)
    total = 0.0
    for c in range(NCORES):
        total += float(np.asarray(res.results[c]["outv"], dtype=np.float64).sum())
    loss = total / float(B * B)
    return np.asarray(loss, dtype=np.float32), res


def kernel(fc_feats_0, fc_feats_1, textual_features, labels):
    loss, _ = run(fc_feats_0, fc_feats_1, textual_features, labels, trace=False)
    return loss


# revision 51
# speedup vs baseline: 1.0860x; 1.0860x over previous
"""Trainium2 Bass kernel for nn_ContrastiveLoss (B=4096, D=1024, 8 cores).

loss = mean over [B,B] of
    labels*(1-sim0) + (1-labels)*relu(sim0-0.5)
  + labels*(1-sim1) + (1-labels)*relu(sim1-0.5)
where sim_k = cos_sim(fc_feats_k[i], textual_features[j]).

Strategy (data-parallel over rows, t replicated -- no collectives):
  * Each of the 8 cores gets a 512-row slice of fc_feats_0/1 and labels
    (bf16 from the host), plus the FULL textual_features (bf16).
  * t is normalized on-chip (x64 fp8 scale); transposes are done by the
    XBAR DMA engine (dma_start_transpose) on bf16 tiles -- no PE
    transposes, no evacuation copies; a single cast pass produces fp8.
  * f0/f1 are NOT normalized: raw bf16 rows are XBAR-transposed and
    cast to fp8; 1/||f_i|| is applied as a per-partition ACT scale in
    the relu pass and factored out of the bilinear sums algebraically.
  * Per S-tile [128,1024] (= 2 j-chunks, one psum tile per f):
      ACT:  r = relu(rin_f * S - 0.5), accum -> racc
      DVE:  accum L * S_raw           -> pacc   (x rin_f in finisher)
      DVE:  w = r0 + r1; accum (w-2)*L -> qacc  (folds C and 2*sum(L))
  * total_core = sum_i [ A - q - rin0*p0 - rin1*p1 ]; host sums / B^2.

Self-contained: hardcodes shapes; only needs concourse + ml_dtypes.
"""

import os
import sys

import numpy as np

B = 4096
D = 1024
NCORES = 8
ROWS = B // NCORES          # 512 rows of f0/f1/labels per core
IT = ROWS // 128            # 4 i-tiles per core
KS = D // 128               # 8 k-subtiles (contraction)
JQ = 4                      # phase-B chunk = 2 slots = 1024 S columns
NSLOT = 8                   # t row-slots of 512 (full t on every core)
MARGIN = 0.5
EPS = 1e-8
TN_SCALE = 64.0             # fp8 scale on normalized t rows

_CACHE = {}

# how many of the 8 t-slot fp8 casts run on ACT (rest on DVE)
# (note: tensor_tensor_reduce fails at runtime on this HW path; squares
# use DVE scalar_tensor_tensor instead)
CASTS_ON_ACT = int(os.environ.get("KERNEL_CASTS_ON_ACT", "6"))


def _import_concourse():
    try:
        import concourse.bass  # noqa: F401
    except ImportError:
        for p in ("/opt/trn_rl_repo", "/root/.axon_site/_ro/trn_rl_repo"):
            if os.path.isdir(p) and p not in sys.path:
                sys.path.insert(0, p)
        import concourse.bass  # noqa: F401


def _build_nc():
    _import_concourse()
    import concourse.bass as bass  # noqa: F401
    import concourse.mybir as mybir
    import concourse.tile as tile
    from concourse import bacc

    f32 = mybir.dt.float32
    bf16 = mybir.dt.bfloat16
    fp8 = mybir.dt.float8e4
    AF = mybir.ActivationFunctionType
    OP = mybir.AluOpType
    AX = mybir.AxisListType
    DR = mybir.MatmulPerfMode.DoubleRow

    nc = bacc.Bacc(
        "TRN2",
        target_bir_lowering=False,
        debug=False,
        num_devices=NCORES,
    )

    f0_d = nc.dram_tensor("f0", [ROWS, D], bf16, kind="ExternalInput").ap()
    f1_d = nc.dram_tensor("f1", [ROWS, D], bf16, kind="ExternalInput").ap()
    tx_d = nc.dram_tensor("tx", [B, D], bf16, kind="ExternalInput").ap()
    lab_d = nc.dram_tensor("lab", [ROWS, B], bf16, kind="ExternalInput").ap()
    out_d = nc.dram_tensor("outv", [128, 1], f32, kind="ExternalOutput").ap()

    with tile.TileContext(nc) as tc:
        with (
            tc.tile_pool(name="constp", bufs=1) as constp,
            tc.tile_pool(name="natp", bufs=5) as natp,
            tc.tile_pool(name="tnbp", bufs=2) as tnbp,
            tc.tile_pool(name="stagep", bufs=3) as stagep,
            tc.tile_pool(name="sqp", bufs=2) as sqp,
            tc.tile_pool(name="small", bufs=6) as small,
            tc.tile_pool(name="fTp", bufs=1) as fTp,
            tc.tile_pool(name="tnTp", bufs=1) as tnTp,
            tc.tile_pool(name="labp", bufs=4) as labp,
            tc.tile_pool(name="rbufp", bufs=4) as rbufp,
            tc.tile_pool(name="scrp", bufs=4) as scrp,
            tc.tile_pool(name="accp", bufs=1) as accp,
            tc.tile_pool(name="mpsum", bufs=4, space="PSUM") as mpsum,
        ):
            negmargin = constp.tile([128, 1], f32)
            nc.gpsimd.memset(negmargin, -MARGIN)

            # persistent per-(f,ic) inverse norms, col = f*IT + ic
            rinp = accp.tile([128, 2 * IT], f32)
            # accumulators, each column written exactly once
            racc = accp.tile([128, 2 * IT * JQ], f32)   # sum relu
            pacc = accp.tile([128, 2 * IT * JQ], f32)   # sum L*Sraw
            qacc = accp.tile([128, IT * JQ], f32)       # sum (w-2)*L

            # ---- input DMAs. t/f stream on the Pool (gpsimd) ring in slot
            # order (the ring's depth-2 + tile-pool recycling paces it);
            # labels ride the Sync ring FIRST (they complete before the
            # first XBAR transpose needs that ring). No compute queue
            # hosts DMA triggers. ----
            Lbs = []
            for ic in range(IT):
                Lb = labp.tile([128, B], bf16, tag="Lb", name=f"Lb_{ic}")
                nc.sync.dma_start(Lb, lab_d[ic * 128:(ic + 1) * 128, :])
                Lbs.append(Lb)

            def t_dma(s):
                t = natp.tile([128, 4, D], bf16, tag="nat", name=f"tnat_{s}")
                nc.gpsimd.dma_start(
                    t, tx_d[s * 512:(s + 1) * 512, :].rearrange(
                        "(r p) d -> p r d", p=128))
                return t

            tnat = {s: t_dma(s) for s in range(2)}
            fnat = []
            for f, src in enumerate((f0_d, f1_d)):
                natb = natp.tile([128, IT, D], bf16, tag="nat",
                                 name=f"fnat_{f}")
                nc.gpsimd.dma_start(
                    natb, src.rearrange("(r p) d -> p r d", p=128))
                fnat.append(natb)
            for s in range(2, NSLOT):
                tnat[s] = t_dma(s)

            def norm_smalls(ssq, n, scale_mul, dst):
                """[128,n] batched: dst = scale_mul / max(sqrt(ssq), EPS)."""
                nrm = small.tile([128, n], f32, tag="nrm",
                                 name=f"nrm_{dst.tensor.name}_{dst.offset}")
                nc.scalar.activation(nrm, ssq, AF.Sqrt)
                nc.vector.tensor_scalar_max(nrm, nrm, EPS)
                nc.vector.reciprocal(dst, nrm)
                nc.vector.tensor_scalar_mul(dst, dst, scale_mul)

            def squares(natb, it, ssq_col, name, on_act):
                """Sum-of-squares of natb[:, it, :] -> ssq_col. ACT Square
                stalls ~1.2us on its accumulator between instructions, so
                early tiles run on DVE (idle then) and later ones on ACT."""
                sqs = sqp.tile([128, D], bf16, tag="sq", name=name)
                if on_act:
                    nc.scalar.activation(sqs, natb[:, it, :], AF.Square,
                                         accum_out=ssq_col)
                else:
                    nc.vector.scalar_tensor_tensor(
                        out=sqs, in0=natb[:, it, :], scalar=1.0,
                        in1=natb[:, it, :], op0=OP.bypass, op1=OP.mult,
                        accum_out=ssq_col)

            def cast_stage(dst8, stage32, on_act):
                """bf16 [128,32,128] xbar stage -> fp8 [128,8,512] with the
                (r ks) -> ks r layout fix folded into the strided read."""
                src = stage32[:, :, :].rearrange("p (r ks) j -> p ks r j",
                                                 ks=KS)
                dst = dst8.rearrange("p ks (r j) -> p ks r j", j=128)
                (nc.scalar.copy if on_act else nc.vector.tensor_copy)(dst, src)

            tnT = tnTp.tile([128, NSLOT * KS, 512], fp8)

            def t_prep(s):
                """Normalize + xbar-transpose + fp8-cast t slot s into tnT."""
                if s not in tnat:
                    tnat[s] = t_dma(s)
                natb = tnat.pop(s)
                ssqs = small.tile([128, 4], f32, tag="ssq", name=f"ssqt_{s}")
                for it in range(4):
                    squares(natb, it, ssqs[:, it:it + 1], f"tsq_{s}_{it}",
                            on_act=(s >= 4))
                rint = small.tile([128, 4], f32, tag="rint", name=f"rint_{s}")
                norm_smalls(ssqs, 4, TN_SCALE, rint)
                tnb = tnbp.tile([128, 4, D], bf16, tag="tnb", name=f"tnb_{s}")
                for it in range(4):
                    nc.vector.tensor_scalar_mul(
                        tnb[:, it, :], natb[:, it, :], rint[:, it:it + 1])
                tstage = stagep.tile([128, 4 * KS, 128], bf16, tag="stage",
                                     name=f"tstage_{s}")
                nc.sync.dma_start_transpose(tstage, tnb[:, :, :])
                # early slots cast on DVE (ACT is busy with squares then);
                # later slots on ACT to balance total load
                cast_stage(tnT[:, s * KS:(s + 1) * KS, :], tstage,
                           on_act=(s >= 3))

            # ---- t slots 0,1 first, then f0/f1 ----
            t_prep(0)
            t_prep(1)
            fTs = []
            for f in range(2):
                natb = fnat[f]
                ssqf = small.tile([128, IT], f32, tag="ssq", name=f"ssqf_{f}")
                for it in range(IT):
                    squares(natb, it, ssqf[:, it:it + 1], f"fsq_{f}_{it}",
                            on_act=True)
                norm_smalls(ssqf, IT, 1.0 / TN_SCALE,
                            rinp[:, f * IT:(f + 1) * IT])
                fstage = stagep.tile([128, IT * KS, 128], bf16, tag="stage",
                                     name=f"fstage_{f}")
                nc.sync.dma_start_transpose(fstage, natb[:, :, :])
                fT = fTp.tile([128, KS, ROWS], fp8, name=f"fT_{f}")
                cast_stage(fT, fstage, on_act=(f == 1))
                fTs.append(fT)

            # ---- slot-pipelined phase B (slots prepped ahead of use) ----
            t_prep(2)
            t_prep(3)
            for jq in range(JQ):
                jsl = slice(jq * 1024, (jq + 1) * 1024)
                for ic in range(IT):
                    isl = slice(ic * 128, (ic + 1) * 128)
                    Lb = Lbs[ic]
                    rbufs = []
                    for f in range(2):
                        fT = fTs[f]
                        ps = mpsum.tile([128, 1024], f32, tag="ps",
                                        name=f"ps_{jq}_{ic}_{f}")
                        for jh in range(2):
                            jc = jq * 2 + jh
                            for k2 in range(KS // 2):
                                ksl = slice(jc * KS + 2 * k2,
                                            jc * KS + 2 * k2 + 2)
                                nc.tensor.matmul(
                                    ps[:, jh * 512:(jh + 1) * 512],
                                    fT[:, 2 * k2:2 * k2 + 2, isl],
                                    tnT[:, ksl, :], perf_mode=DR,
                                    start=(k2 == 0), stop=(k2 == KS // 2 - 1),
                                )
                        g = f * IT + ic
                        rsc = rinp[:, g:g + 1]
                        rb = rbufp.tile([128, 1024], bf16, tag="rb",
                                        name=f"rb_{jq}_{ic}_{f}")
                        pi = (ic * JQ + jq) * 2 + f
                        nc.scalar.activation(rb, ps, AF.Relu, bias=negmargin,
                                             scale=rsc,
                                             accum_out=racc[:, pi:pi + 1])
                        rbufs.append(rb)
                        scr = scrp.tile([128, 1024], bf16, tag="scr",
                                        name=f"scr_{jq}_{ic}_{f}")
                        qi = g * JQ + jq
                        nc.vector.scalar_tensor_tensor(
                            out=scr, in0=ps, scalar=1.0, in1=Lb[:, jsl],
                            op0=OP.bypass, op1=OP.mult,
                            accum_out=pacc[:, qi:qi + 1])
                    # w = r0 + r1 via stt (all-bf16 -> 4x); (w-2)*L accum
                    w = rbufp.tile([128, 1024], bf16, tag="w",
                                   name=f"w_{jq}_{ic}")
                    nc.vector.scalar_tensor_tensor(
                        out=w, in0=rbufs[0], scalar=0.0, in1=rbufs[1],
                        op0=OP.add, op1=OP.add)
                    scr2 = scrp.tile([128, 1024], bf16, tag="scr2",
                                     name=f"scr2_{jq}_{ic}")
                    wi = ic * JQ + jq
                    nc.vector.scalar_tensor_tensor(
                        out=scr2, in0=w, scalar=2.0, in1=Lb[:, jsl],
                        op0=OP.subtract, op1=OP.mult,
                        accum_out=qacc[:, wi:wi + 1])
                # prep upcoming t slots between consumption blocks
                for s in (2 * jq + 4, 2 * jq + 5):
                    if s < NSLOT:
                        t_prep(s)


            # ---- finisher ----
            pred = accp.tile([128, 2 * IT], f32)
            for g in range(2 * IT):
                nc.vector.reduce_sum(pred[:, g:g + 1],
                                     pacc[:, g * JQ:(g + 1) * JQ], axis=AX.X)
            scaled = accp.tile([128, 2 * IT], f32)
            nc.vector.tensor_mul(scaled, pred, rinp)
            ps_tot = small.tile([128, 1], f32, tag="fin", name="ps_tot")
            nc.vector.reduce_sum(ps_tot, scaled, axis=AX.X)
            ar = small.tile([128, 1], f32, tag="fin", name="ar")
            nc.vector.reduce_sum(ar, racc, axis=AX.X)
            qr = small.tile([128, 1], f32, tag="fin", name="qr")
            nc.vector.reduce_sum(qr, qacc, axis=AX.X)
            tmp = small.tile([128, 1], f32, tag="fin", name="tmp")
            nc.vector.tensor_sub(tmp, ar, qr)
            ov = small.tile([128, 1], f32, tag="fin", name="ov")
            nc.vector.tensor_sub(ov, tmp, ps_tot)
            nc.sync.dma_start(out_d, ov)

    nc.compile()
    return nc


def _get_nc():
    if "nc" not in _CACHE:
        _CACHE["nc"] = _build_nc()
    return _CACHE["nc"]


def _make_in_maps(fc_feats_0, fc_feats_1, textual_features, labels):
    import ml_dtypes
    b16 = ml_dtypes.bfloat16
    tx = np.ascontiguousarray(textual_features).astype(b16)
    in_maps = []
    for c in range(NCORES):
        sl = slice(c * ROWS, (c + 1) * ROWS)
        in_maps.append({
            "f0": np.ascontiguousarray(fc_feats_0[sl]).astype(b16),
            "f1": np.ascontiguousarray(fc_feats_1[sl]).astype(b16),
            "tx": tx,
            "lab": np.ascontiguousarray(labels[sl]).astype(b16),
        })
    return in_maps


def run(fc_feats_0, fc_feats_1, textual_features, labels, trace=False):
    """Run on 8 NeuronCores; returns (loss_scalar, BassKernelResults)."""
    _import_concourse()
    from concourse.bass_utils import run_bass_kernel_spmd

    nc = _get_nc()
    in_maps = _make_in_maps(np.asarray(fc_feats_0), np.asarray(fc_feats_1),
                            np.asarray(textual_features), np.asarray(labels))
    res = run_bass_kernel_spmd(nc, in_maps, list(range(NCORES)), trace=trace)
    total = 0.0
    for c in range(NCORES):
        total += float(np.asarray(res.results[c]["outv"], dtype=np.float64).sum())
    loss = total / float(B * B)
    return np.asarray(loss, dtype=np.float32), res


def kernel(fc_feats_0, fc_feats_1, textual_features, labels):
    loss, _ = run(fc_feats_0, fc_feats_1, textual_features, labels, trace=False)
    return loss
